# revision 16
# baseline (speedup 1.0000x reference)
"""AttentionPooling Trainium2 kernel (8 NeuronCores, Bass/Tile).

Sharding: (batch, head-group) — core c handles batch b=c//2 and heads
4*(c%2)..4*(c%2)+3. Each core computes, for its 4 heads, Q^T/K^T (head-dim
major) projections, then a one-pass pooled attention over 64 query stripes
(4 heads x 16 stripes of 128 queries):

  S = Q_stripe K^T / sqrt(d)      (PE, bf16, 4 matmuls into 4 PSUM banks)
  E = exp(S), Z = rowsum(E)       (ScalarE: ONE 2048-wide ACTIVATE + accum)
  r = 1/Z                         (VectorE)
  w_stripe = r^T E                (PE, 4 col-tiled matmuls into 1 bank)
  w_acc += w_stripe               (VectorE f32 add, PSUM->SBUF)

PSUM is managed as one manually-rotated [128, 8, 512] f32 ring:
  - stripe i's scores live in banks 4*(i%2)..+3, so each stripe's exp is a
    single contiguous 2048-wide ACTIVATE (one READ_ACCUMULATOR instead of
    two: ~0.5us/stripe saved on the ScalarE critical path).
  - the stripe's w matmuls reuse bank 4*(i%2) right after the ACTIVATE
    frees it (transient accumulator, k-chunk j at partition offset 32j),
    then VectorE folds it into an SBUF f32 accumulator.
  - interleaved Q/K projection chunks borrow the other 6 banks round-robin.

The V projection is never materialized: attended_mean*N = (w @ x) @ Wv_h^T
(+ bv folded on the host), so the tail computes u = w @ x (16 matmuls
against token-major x) and u @ Wv^T. The mean-pool is folded through the
output projection; V/output biases fold on the host:
  pooled = pooled_partial(core even) + pooled_partial(core odd) + Wo@bv + bo

Non-critical DMAs (remaining heads' Q/K weights, token-major x, Wv, Wo) are
dependency-gated on the first ACTIVATE so the prologue's HBM bandwidth all
goes to x^T + head-0 weights (first exp starts ~20us in instead of ~34us).
"""

import sys

import numpy as np

for _p in ("/opt/trn_rl_repo",):
    if _p not in sys.path:
        sys.path.append(_p)

import ml_dtypes

B, N, HID = 4, 2048, 1024
HEADS, HD = 8, 128
NH = 4          # heads per core
HGW = NH * HD   # head-group width (512)
NCORES = 8
P = 128
IT = HID // P   # 8 i-tiles
QT_TILES = N // P    # 16 query stripes
TOK_TILES = N // P   # 16 token tiles

BF16 = ml_dtypes.bfloat16

_cache = {}


def _build_nc():
    import concourse.bacc as bacc
    import concourse.tile as tile
    from concourse import mybir
    from concourse.bass import ds, ts
    from concourse.masks import make_identity
    from concourse.tile import add_dep_helper

    BF = mybir.dt.bfloat16
    F32 = mybir.dt.float32
    AF = mybir.ActivationFunctionType

    nc = bacc.Bacc(trn_type="TRN2")

    xT_d = nc.dram_tensor("xT", (HID, N), BF, kind="ExternalInput").ap()
    xtok_d = nc.dram_tensor("xtok", (N, HID), BF, kind="ExternalInput").ap()
    wqkT_d = nc.dram_tensor("wqkT", (NH, P, 2, IT, HD), BF, kind="ExternalInput").ap()
    wvT_d = nc.dram_tensor("wvT", (HID, HGW), BF, kind="ExternalInput").ap()
    woT_d = nc.dram_tensor("woT", (HGW, HID), BF, kind="ExternalInput").ap()
    bqk_d = nc.dram_tensor("bqk_col", (P, 2, NH), F32, kind="ExternalInput").ap()
    out_d = nc.dram_tensor("out_pooled", (1, HID), F32, kind="ExternalOutput").ap()

    inv_sqrt_d = float(1.0 / np.sqrt(HD))

    with tile.TileContext(nc) as tc:
        with (
            tc.tile_pool(name="persist", bufs=1) as persist,
            tc.tile_pool(name="ring", bufs=1, space="PSUM") as ringp,
            tc.tile_pool(name="ep", bufs=3) as ep,
            tc.tile_pool(name="zp", bufs=4) as zp,
        ):
            # ---- critical-path DMAs (everything else is gated on act0) ----
            # per-DMA issue overhead is ~0.7us, so the critical set is packed
            # into 6 transfers: combined head-0 Q+K weights, 4 x^T quarters,
            # combined biases.
            xT_sb = persist.tile([P, IT, N], BF)
            wqk_sb = persist.tile([P, NH, 2, IT, HD], BF)
            xT_r = xT_d.rearrange("(t p) n -> p t n", p=P)
            bqk_sb = persist.tile([P, 2, NH], F32)
            nc.sync.dma_start(out=wqk_sb[:, 0], in_=wqkT_d[0])
            # x^T in four 1MiB quarters: early quarters' projection matmuls
            # run while the later quarters transfer
            for qq in range(4):
                nc.sync.dma_start(
                    out=xT_sb[:, 2 * qq : 2 * qq + 2, :],
                    in_=xT_r[:, 2 * qq : 2 * qq + 2, :],
                )
            nc.sync.dma_start(out=bqk_sb, in_=bqk_d)
            # tiles for the gated DMAs (emitted inside the stripe loop)
            xtok_sb = persist.tile([P, TOK_TILES, HID], BF)
            wv_sb = persist.tile([P, IT, HGW], BF)
            wo_sb = persist.tile([P, NH, HID], BF)

            identB = persist.tile([NH, NH], BF)
            make_identity(nc, identB)
            # one-hot columns: oneh_sb[p, h, h'] = 1.0 iff h == h'
            oneh_sb = persist.tile([P, NH, NH], BF)
            nc.vector.memset(oneh_sb, 0.0)
            for h in range(NH):
                nc.vector.memset(oneh_sb[:, h, h : h + 1], 1.0)

            QT_sb = persist.tile([P, NH, N], BF)
            KT_sb = persist.tile([P, NH, N], BF)
            # w accumulator, packed: k-chunk j on partitions 32j..32j+3
            w_acc = persist.tile([P, 512], F32)
            nc.vector.memset(w_acc, 0.0)
            w4_sb = persist.tile([NH, N], BF)
            # wT4[p, t, h] = w_h[t*128+p]  (token-major w for the u matmuls)
            wT4_sb = persist.tile([P, TOK_TILES, NH], BF)
            u4_sb = persist.tile([NH, HID], BF)
            # uTz[p, i, h, h'] = u_h[i*128+p] iff h' == h else 0 (block-diag
            # zero padding so per-head u@Wv^T matmuls share one accumulator)
            uTz_sb = persist.tile([P, IT, NH, NH], BF)
            nc.vector.memset(uTz_sb, 0.0)
            att4_sb = persist.tile([NH, P], BF)
            attT_sb = persist.tile([P, NH], BF)
            pooled_sb = persist.tile([1, HID], F32)

            # ---- the 8-bank PSUM ring ----
            R = ringp.tile([P, 8, 512], F32, name="ring")

            def qk_chunk(proj_i, h, c, ps):
                """One full 512-token Q^T/K^T projection chunk (prologue
                only) into the given PSUM bank AP."""
                dst = (QT_sb, KT_sb)[proj_i]
                for i in range(IT):
                    nc.tensor.matmul(
                        ps,
                        lhsT=wqk_sb[:, h, proj_i, i, :],
                        rhs=xT_sb[:, i, ts(c, 512)],
                        start=(i == 0),
                        stop=(i == IT - 1),
                    )
                nc.vector.tensor_scalar_add(
                    dst[:, h, ts(c, 512)], ps, bqk_sb[:, proj_i, h : h + 1]
                )

            # ---------------- prologue: head 0's K + first Q chunk --------
            # K chunks in banks 4..7 (stripe 1's group), Q chunk in bank 0.
            # Matmuls are emitted grouped by i-tile so they chase the x^T
            # quarter DMAs instead of serializing each chunk behind the last
            # quarter; evac+bias runs on the (otherwise idle) ScalarE.
            PRO = ((1, 0, 0, 4), (1, 0, 1, 5), (1, 0, 2, 6), (1, 0, 3, 7),
                   (0, 0, 0, 0))
            for i in range(IT):
                for pi, h0, c, bank in PRO:
                    nc.tensor.matmul(
                        R[:, bank, :],
                        lhsT=wqk_sb[:, h0, pi, i, :],
                        rhs=xT_sb[:, i, ts(c, 512)],
                        start=(i == 0),
                        stop=(i == IT - 1),
                    )
            for pi, h0, c, bank in (PRO[4], PRO[1], PRO[2], PRO[3], PRO[0]):
                dst = (QT_sb, KT_sb)[pi]
                nc.scalar.add(
                    dst[:, h0, ts(c, 512)], R[:, bank, :], bqk_sb[:, pi, h0 : h0 + 1]
                )

            # Background projection work: remaining heads' Q/K chunks, split
            # into 4-matmul HALF-chunks (contraction i-tiles 0-3 / 4-7), one
            # or two halves per stripe. Each half is an atomic PSUM
            # accumulation group in a bank of group si%2 (excluding that
            # group's w bank): in program order that bank was last read by
            # ACT(si) and is next score-written by emit_S(si+2), which is
            # emitted after bg_advance(si) — no writer interleaving. Halving
            # the chunks keeps the PE lump (and its trailing DVE evac) small
            # enough that the w-evac -> S(kc0) chain never misses its slot.
            bg_n = [0]
            bg_q = []
            specs = []
            for c in range(1, 4):
                specs.append((0, 0, c))
            for h2 in range(1, NH):
                # K chunks first: head h2's stripes start at stripe 16*h2
                # and need ALL of K^T(h2) but only the first Q chunk.
                for c in range(4):
                    specs.append((1, h2, c))
                for c in range(4):
                    specs.append((0, h2, c))
            for spec in specs:
                bg_q.append((*spec, 0))
                bg_q.append((*spec, 1))
            bg_q.reverse()

            def bg_advance(si):
                for _ in range(2 if si < 10 else 1):
                    if not bg_q:
                        return
                    proj_i, h2, c, half = bg_q.pop()
                    bank = 4 * (si % 2) + 1 + bg_n[0] % 3
                    bg_n[0] += 1
                    ps = R[:, bank, :]
                    dst = (QT_sb, KT_sb)[proj_i]
                    for ii in range(4 * half, 4 * half + 4):
                        nc.tensor.matmul(
                            ps,
                            lhsT=wqk_sb[:, h2, proj_i, ii, :],
                            rhs=xT_sb[:, ii, ts(c, 512)],
                            start=(ii % 4 == 0),
                            stop=(ii % 4 == 3),
                        )
                    if half == 0:
                        nc.vector.tensor_scalar_add(
                            dst[:, h2, ts(c, 512)], ps, bqk_sb[:, proj_i, h2 : h2 + 1]
                        )
                    else:
                        nc.vector.tensor_tensor(
                            dst[:, h2, ts(c, 512)],
                            dst[:, h2, ts(c, 512)],
                            ps,
                            mybir.AluOpType.add,
                        )

            # ---------------- pooled attention stripe loop ----------------
            def emit_S(h, qi, grp):
                # kc=0's bank doubles as the previous stripe's transient w
                # accumulator; emit it last so it trails that w evacuation
                for kc in (1, 2, 3, 0):
                    nc.tensor.matmul(
                        R[:, 4 * grp + kc, :],
                        lhsT=QT_sb[:, h, ts(qi, P)],
                        rhs=KT_sb[:, h, ds(kc * 512, 512)],
                        start=True,
                        stop=True,
                    )

            def emit_w(e_t, rb4_t, b0):
                for j in range(4):
                    # each k-chunk region is written by exactly ONE matmul
                    # (own start/stop group): start=True's has_written clear
                    # must not let a sibling region accumulate stale scores
                    nc.tensor.matmul(
                        R[32 * j : 32 * j + NH, b0, :],
                        lhsT=rb4_t,
                        rhs=e_t[:, ts(j, 512)],
                        start=True,
                        stop=True,
                        tile_position=(0, 32 * j) if j else None,
                        skip_group_check=True,
                    )
                nc.vector.tensor_tensor(
                    w_acc, w_acc, R[:, b0, :], mybir.AluOpType.add
                )

            NSTRIPES = NH * QT_TILES
            emit_S(0, 0, 0)
            for i in range(NSTRIPES):
                h, qi = i // QT_TILES, i % QT_TILES
                b0 = 4 * (i % 2)
                e_t = ep.tile([P, N], BF, tag="e", name="e_t")
                z_t = zp.tile([P, 1], F32, tag="z", name="z_t")
                act = nc.scalar.activation(
                    out=e_t,
                    in_=R[:, b0 : b0 + 4, :].rearrange("p a b -> p (a b)"),
                    func=AF.Exp,
                    scale=inv_sqrt_d,
                    accum_out=z_t,
                )
                if i == 0:
                    # non-critical DMAs, gated so they don't steal prologue
                    # HBM bandwidth from x^T / head-0 weights
                    gated = []
                    for h2 in range(1, NH):
                        gated.append(
                            nc.sync.dma_start(out=wqk_sb[:, h2], in_=wqkT_d[h2])
                        )
                    gated.append(
                        nc.sync.dma_start(
                            out=xtok_sb,
                            in_=xtok_d.rearrange("(t p) d -> p t d", p=P),
                        )
                    )
                    gated.append(
                        nc.sync.dma_start(
                            out=wv_sb, in_=wvT_d.rearrange("(t p) d -> p t d", p=P)
                        )
                    )
                    gated.append(
                        nc.sync.dma_start(
                            out=wo_sb, in_=woT_d.rearrange("(t p) o -> p t o", p=P)
                        )
                    )
                    for g in gated:
                        add_dep_helper(g.ins, act.ins, sync=True, reason="defer-dma")
                if i + 1 < NSTRIPES:
                    ni = i + 1
                    emit_S(ni // QT_TILES, ni % QT_TILES, ni % 2)
                r_t = zp.tile([P, 1], F32, tag="r", name="r_t")
                nc.vector.reciprocal(r_t, z_t)
                # rb4 column h = r (bf16), other columns zero
                rb4_t = zp.tile([P, NH], BF, tag="rb", name="rb4_t")
                nc.vector.tensor_tensor(
                    rb4_t,
                    oneh_sb[:, h, :],
                    r_t.to_broadcast((P, NH)),
                    mybir.AluOpType.mult,
                )
                # this stripe's w matmuls: emitted after S(i+1) (so their
                # ACT(i)-end wait never blocks scores) but before the
                # background chunk (so the chunk's PE lump and DVE evac
                # trail the w-evac -> S(kc0) critical chain, not lead it)
                emit_w(e_t, rb4_t, b0)
                # interleaved background projection work
                bg_advance(i)

            # ---------------- tail ----------------
            # w_acc (packed f32) -> w4_sb [4, 2048] bf16
            for j in range(4):
                nc.vector.tensor_copy(w4_sb[:, ts(j, 512)], w_acc[32 * j : 32 * j + NH, :])

            def psum_bf(bank, n_bf):
                """bf16 view of ring bank columns [0, n_bf) -> [P, n_bf]."""
                return R[:, bank, 0 : (n_bf + 1) // 2].bitcast(BF)

            # pipelined: transpose w4 chunk t -> wT4, then its two u matmuls
            # (u = w @ x accumulated in banks 2 and 3)
            for t in range(TOK_TILES):
                tpps = psum_bf(5 + t % 3, NH)  # [P, 4] bf16
                nc.tensor.transpose(tpps, w4_sb[:, ts(t, P)], identB)
                nc.vector.tensor_copy(wT4_sb[:, t, :], tpps)
                for dc in range(2):
                    nc.tensor.matmul(
                        R[0:NH, 2 + dc, :],
                        lhsT=wT4_sb[:, t, :],
                        rhs=xtok_sb[:, t, ts(dc, 512)],
                        start=(t == 0),
                        stop=(t == TOK_TILES - 1),
                    )
            for dc in range(2):
                nc.vector.tensor_copy(u4_sb[:, ts(dc, 512)], R[0:NH, 2 + dc, :])
            # pipelined: transpose u chunk i -> uTz (block-diag scatter), then
            # its 4 att matmuls (att4 = u @ Wv^T accumulated in bank 1)
            for i in range(IT):
                tpps = psum_bf(5 + i % 3, NH)
                nc.tensor.transpose(tpps, u4_sb[:, ts(i, P)], identB)
                nc.vector.tensor_copy(
                    uTz_sb[:, i].rearrange("p a b -> p (a b)")[:, :: NH + 1],
                    tpps,
                )
                for h in range(NH):
                    nc.tensor.matmul(
                        R[0:NH, 1, 0:P],
                        lhsT=uTz_sb[:, i, h, :],
                        rhs=wv_sb[:, i, ts(h, HD)],
                        start=(i == 0 and h == 0),
                        stop=(i == IT - 1 and h == NH - 1),
                    )
            nc.vector.tensor_copy(att4_sb, R[0:NH, 1, 0:P])
            attT_ps = psum_bf(0, NH)
            nc.tensor.transpose(attT_ps, att4_sb, identB)
            nc.vector.tensor_copy(attT_sb, attT_ps)
            for oc in range(2):
                for h in range(NH):
                    nc.tensor.matmul(
                        R[0:1, 4 + oc, :],
                        lhsT=attT_sb[:, h : h + 1],
                        rhs=wo_sb[:, h, ts(oc, 512)],
                        start=(h == 0),
                        stop=(h == NH - 1),
                    )
            for oc in range(2):
                nc.vector.tensor_copy(pooled_sb[:, ts(oc, 512)], R[0:1, 4 + oc, :])
            nc.sync.dma_start(out=out_d, in_=pooled_sb)

    nc.finalize()  # Bacc: event-sem pass packs multi-waits into legal encodings
    return nc


def _get_nc():
    if "nc" not in _cache:
        _cache["nc"] = _build_nc()
    return _cache["nc"]


def _host_prep(inputs):
    """Build the 8 per-core input maps (host-side shard + transpose + cast)."""
    x = np.asarray(inputs["chunk_embeddings"], np.float32)
    in_maps = []
    for c in range(NCORES):
        b, hg = c // 2, c % 2
        sl = slice(hg * HGW, (hg + 1) * HGW)
        in_maps.append(
            {
                "xT": np.ascontiguousarray(x[b].T).astype(BF16),
                "xtok": np.ascontiguousarray(x[b]).astype(BF16),
                "wqkT": np.ascontiguousarray(
                    np.stack(
                        [
                            np.asarray(inputs[k], np.float32)[sl, :]
                            .T.reshape(HID, NH, HD)
                            .transpose(1, 0, 2)
                            for k in ("Wq", "Wk")
                        ]
                    )  # [2, NH, HID, HD]
                    .reshape(2, NH, IT, P, HD)
                    .transpose(1, 3, 0, 2, 4)  # [NH, P, 2, IT, HD]
                ).astype(BF16),
                "wvT": np.ascontiguousarray(
                    np.asarray(inputs["Wv"], np.float32)[sl, :].T
                ).astype(BF16),
                "woT": np.ascontiguousarray(
                    np.asarray(inputs["Wo"], np.float32)[:, sl].T / np.float32(N)
                ).astype(BF16),
                "bqk_col": np.ascontiguousarray(
                    np.stack(
                        [
                            np.asarray(inputs[k], np.float32)[sl].reshape(NH, P).T
                            for k in ("bq", "bk")
                        ],
                        axis=1,
                    )
                ),
            }
        )
    return in_maps


def _unshard(results, inputs):
    bo = np.asarray(inputs["bo"], np.float32)
    bv = np.asarray(inputs["bv"], np.float32)
    Wo = np.asarray(inputs["Wo"], np.float32)
    bv_wo = Wo @ bv  # exact fold of the V bias through the output projection
    out = np.zeros((B, HID), np.float32)
    for b in range(B):
        out[b] = (
            results[2 * b]["out_pooled"][0]
            + results[2 * b + 1]["out_pooled"][0]
            + bv_wo
            + bo
        )
    return out


def _reference_numpy(inputs):
    """Fallback for non-trivial attention masks (never hit for the spec'd
    all-ones mask): straight numpy port of the reference."""
    x = np.asarray(inputs["chunk_embeddings"], np.float32)
    mask = np.asarray(inputs["attention_mask"])
    b, n, hid = x.shape

    def proj(W, bias):
        y = x @ np.asarray(W, np.float32).T + np.asarray(bias, np.float32)
        return y.reshape(b, n, HEADS, HD).transpose(0, 2, 1, 3)

    Q = proj(inputs["Wq"], inputs["bq"])
    K = proj(inputs["Wk"], inputs["bk"])
    V = proj(inputs["Wv"], inputs["bv"])
    s = np.einsum("bhqd,bhkd->bhqk", Q, K) / np.float32(np.sqrt(HD))
    s = np.where(mask[:, None, None, :] == 0, np.float32(-1e9), s)
    s = s - s.max(axis=-1, keepdims=True)
    e = np.exp(s)
    a = e / e.sum(axis=-1, keepdims=True)
    att = np.einsum("bhqk,bhkd->bhqd", a, V)
    att = att.transpose(0, 2, 1, 3).reshape(b, n, hid)
    out = att @ np.asarray(inputs["Wo"], np.float32).T + np.asarray(
        inputs["bo"], np.float32
    )
    m = mask[:, :, None].astype(np.float32)
    return (out * m).sum(axis=1) / m.sum(axis=1)


def _run(inputs, trace=False):
    from concourse.bass_utils import run_bass_kernel_spmd

    nc = _get_nc()
    in_maps = _host_prep(inputs)
    res = run_bass_kernel_spmd(
        nc, in_maps, core_ids=list(range(NCORES)), trace=trace
    )
    _cache["last_result"] = res
    return _unshard(res.results, inputs)


def kernel(**inputs):
    mask = np.asarray(inputs["attention_mask"])
    if not np.all(mask == 1):
        return _reference_numpy(inputs)
    return _run(inputs, trace=False)


def kernel_traced(**inputs):
    """Like kernel() but with NTFF profiling; returns (out, exec_time_ns)."""
    out = _run(inputs, trace=True)
    return out, _cache["last_result"].exec_time_ns


# revision 17
# speedup vs baseline: 1.0259x; 1.0259x over previous
"""AttentionPooling Trainium2 kernel (8 NeuronCores, Bass/Tile).

Sharding: (batch, head-group) — core c handles batch b=c//2 and heads
4*(c%2)..4*(c%2)+3. Each core computes, for its 4 heads, Q^T/K^T (head-dim
major) projections, then a one-pass pooled attention over 64 query stripes
(4 heads x 16 stripes of 128 queries):

  S = Q_stripe K^T / sqrt(d)      (PE, bf16, 4 matmuls, two [128,1024] tiles)
  E = exp(S), Z0/Z1 = rowsums     (ScalarE: two 1024-wide ACTIVATEs + accum)
  r = 1/(Z0+Z1)                   (VectorE)
  w += r^T E                      (PE, 4 col-tiled matmuls, PSUM-accumulated)

PSUM budget (8 banks of 512 fp32): sp pool 3x[128,1024] f32 S half-tiles
(6 banks, 1.5-stripe-deep softmax pipeline), wp 1 bank (w accumulator,
k-chunk j at partition offset 32j via matmul column tile_position), pp 1
bank (projection chunks). The w/proj banks are disjoint from the S ring, so
the exp pipeline's only dependency is its own score matmuls.

The V projection is never materialized: attended_mean*N = (w @ x) @ Wv_h^T
(+ bv folded on the host), so the tail computes u = w @ x against
token-major x and u @ Wv^T, with single-pass bf16 transposes pipelined
against the u/att matmuls. The mean-pool is folded through the output
projection; V/output biases fold on the host:
  pooled = pooled_partial(core even) + pooled_partial(core odd) + Wo@bv + bo

Prologue: per-DMA issue is ~0.7us, so the critical set is 6 transfers
(combined head-0 Q+K weights, 4 x^T quarters, combined biases); head-0
projection matmuls are emitted grouped by contraction i-tile so they chase
the x^T quarters, and their evac+bias runs on the (otherwise idle) ScalarE.
All other DMAs are dependency-gated on the first ACTIVATE.
"""

import sys

import numpy as np

for _p in ("/opt/trn_rl_repo",):
    if _p not in sys.path:
        sys.path.append(_p)

import ml_dtypes

B, N, HID = 4, 2048, 1024
HEADS, HD = 8, 128
NH = 4          # heads per core
HGW = NH * HD   # head-group width (512)
NCORES = 8
P = 128
IT = HID // P   # 8 i-tiles
QT_TILES = N // P    # 16 query stripes
TOK_TILES = N // P   # 16 token tiles

BF16 = ml_dtypes.bfloat16

_cache = {}


def _build_nc():
    import concourse.bacc as bacc
    import concourse.tile as tile
    from concourse import mybir
    from concourse.bass import ds, ts
    from concourse.masks import make_identity
    from concourse.tile import add_dep_helper

    BF = mybir.dt.bfloat16
    F32 = mybir.dt.float32
    AF = mybir.ActivationFunctionType

    nc = bacc.Bacc(trn_type="TRN2")

    xT_d = nc.dram_tensor("xT", (HID, N), BF, kind="ExternalInput").ap()
    xtok_d = nc.dram_tensor("xtok", (N, HID), BF, kind="ExternalInput").ap()
    wqkT_d = nc.dram_tensor("wqkT", (NH, P, 2, IT, HD), BF, kind="ExternalInput").ap()
    wvT_d = nc.dram_tensor("wvT", (HID, HGW), BF, kind="ExternalInput").ap()
    woT_d = nc.dram_tensor("woT", (HGW, HID), BF, kind="ExternalInput").ap()
    bqk_d = nc.dram_tensor("bqk_col", (P, 2, NH), F32, kind="ExternalInput").ap()
    out_d = nc.dram_tensor("out_pooled", (1, HID), F32, kind="ExternalOutput").ap()

    inv_sqrt_d = float(1.0 / np.sqrt(HD))

    with tile.TileContext(nc) as tc:
        with (
            tc.tile_pool(name="persist", bufs=1) as persist,
            tc.tile_pool(name="sp", bufs=3, space="PSUM") as sp,
            tc.tile_pool(name="wp", bufs=1, space="PSUM") as wp,
            tc.tile_pool(name="pp", bufs=1, space="PSUM") as pp,
            tc.tile_pool(name="ep", bufs=3) as ep,
            tc.tile_pool(name="zp", bufs=4) as zp,
        ):
            # ---- critical-path DMAs (everything else is gated on act0) ----
            xT_sb = persist.tile([P, IT, N], BF)
            wqk_sb = persist.tile([P, NH, 2, IT, HD], BF)
            xT_r = xT_d.rearrange("(t p) n -> p t n", p=P)
            bqk_sb = persist.tile([P, 2, NH], F32)
            nc.sync.dma_start(out=wqk_sb[:, 0], in_=wqkT_d[0])
            for qq in range(4):
                nc.sync.dma_start(
                    out=xT_sb[:, 2 * qq : 2 * qq + 2, :],
                    in_=xT_r[:, 2 * qq : 2 * qq + 2, :],
                )
            nc.sync.dma_start(out=bqk_sb, in_=bqk_d)
            # tiles for the gated DMAs (emitted inside the stripe loop)
            xtok_sb = persist.tile([P, TOK_TILES, HID], BF)
            wv_sb = persist.tile([P, IT, HGW], BF)
            wo_sb = persist.tile([P, NH, HID], BF)

            identB = persist.tile([NH, NH], BF)
            make_identity(nc, identB)
            # one-hot columns: oneh_sb[p, h, h'] = 1.0 iff h == h'
            oneh_sb = persist.tile([P, NH, NH], BF)
            nc.vector.memset(oneh_sb, 0.0)
            for h in range(NH):
                nc.vector.memset(oneh_sb[:, h, h : h + 1], 1.0)
            zs4_sb = persist.tile([P, NH], BF)
            nc.vector.memset(zs4_sb, 0.0)

            QT_sb = persist.tile([P, NH, N], BF)
            KT_sb = persist.tile([P, NH, N], BF)
            w4_sb = persist.tile([NH, N], BF)
            wT4_sb = persist.tile([P, TOK_TILES, NH], BF)
            u4_sb = persist.tile([NH, HID], BF)
            # uTz[p, i, h, h'] = u_h[i*128+p] iff h' == h else 0 (block-diag
            # zero padding so per-head u@Wv^T matmuls share one accumulator)
            uTz_sb = persist.tile([P, IT, NH, NH], BF)
            nc.vector.memset(uTz_sb, 0.0)
            att4_sb = persist.tile([NH, P], BF)
            attT_sb = persist.tile([P, NH], BF)
            pooled_sb = persist.tile([1, HID], F32)

            # ---------------- prologue: head 0's K + first Q chunk --------
            # 5 parallel [128,512] chunk accumulators: 3 borrowed sp slots +
            # wp + pp. Matmuls grouped by i-tile chase the x^T quarters;
            # evac+bias on ScalarE (idle until the first exp).
            pro_ps = [
                sp.tile([P, 512], F32, tag="s", name="pro0"),
                sp.tile([P, 512], F32, tag="s", name="pro1"),
                sp.tile([P, 512], F32, tag="s", name="pro2"),
                wp.tile([P, 512], F32, tag="w", name="pro3"),
                pp.tile([P, 512], F32, tag="proj", name="pro4"),
            ]
            # (proj_i, chunk): K c0..c3 then Q c0
            PRO = ((1, 0), (1, 1), (1, 2), (1, 3), (0, 0))
            for i in range(IT):
                for (pi, c), ps in zip(PRO, pro_ps):
                    nc.tensor.matmul(
                        ps,
                        lhsT=wqk_sb[:, 0, pi, i, :],
                        rhs=xT_sb[:, i, ts(c, 512)],
                        start=(i == 0),
                        stop=(i == IT - 1),
                    )
            # evac order: Q first (stripe 0's lhsT), then K c0..c3
            for (pi, c), ps in [(PRO[4], pro_ps[4])] + list(zip(PRO[:4], pro_ps[:4])):
                dst = (QT_sb, KT_sb)[pi]
                nc.scalar.add(dst[:, 0, ts(c, 512)], ps, bqk_sb[:, pi, 0:1])

            # Background projection work: remaining heads' Q/K chunks in the
            # dedicated pp bank; generators yield mid-chunk for fine-grained
            # interleaving (safe: nothing else writes the pp bank, and the
            # pool serializes chunk n+1 behind chunk n's evacuation).
            def qk_chunk(proj_i, h, c):
                dst = (QT_sb, KT_sb)[proj_i]
                ps = pp.tile([P, 512], F32, tag="proj", name="ps_qk")
                for i in range(IT):
                    nc.tensor.matmul(
                        ps,
                        lhsT=wqk_sb[:, h, proj_i, i, :],
                        rhs=xT_sb[:, i, ts(c, 512)],
                        start=(i == 0),
                        stop=(i == IT - 1),
                    )
                    if i == 3:
                        yield
                nc.vector.tensor_scalar_add(
                    dst[:, h, ts(c, 512)], ps, bqk_sb[:, proj_i, h : h + 1]
                )
                yield

            bg_tasks = []
            for c in range(1, 4):
                bg_tasks.append(qk_chunk(0, 0, c))
            for h2 in range(1, NH):
                # K chunks first: head h2's stripes start at stripe 16*h2
                # and need ALL of K^T(h2) but only the first Q chunk.
                for c in range(4):
                    bg_tasks.append(qk_chunk(1, h2, c))
                for c in range(4):
                    bg_tasks.append(qk_chunk(0, h2, c))
            bg_tasks.reverse()
            BG_STEPS = 2 * len(bg_tasks)
            BG_SPREAD = 40  # finish all background work by stripe 40 of 64

            def bg_advance(si):
                lo = si * BG_STEPS // BG_SPREAD
                hi = min((si + 1) * BG_STEPS // BG_SPREAD, BG_STEPS)
                for _ in range(max(0, hi - lo)):
                    while bg_tasks:
                        try:
                            next(bg_tasks[-1])
                            break
                        except StopIteration:
                            bg_tasks.pop()

            # ---------------- pooled attention stripe loop ----------------
            # w accumulator: [128, 512] fp32 = 1 PSUM bank. k-chunk j lives
            # at partitions [32j, +4) (heads on rows +0..3) via matmul column
            # tile_position. Zero-matmuls open each sub-region's accumulation
            # group so later matmuls can all use start=False regardless of
            # has_written clear granularity.
            w4_ps = wp.tile([P, 512], F32, tag="w", name="w4_ps")

            def w_region(j):
                poff = 32 * j
                out = w4_ps[poff : poff + NH, :]
                tp = (0, poff) if poff else None
                return out, tp

            for j in range(4):
                out, tp = w_region(j)
                nc.tensor.matmul(
                    out,
                    lhsT=zs4_sb,
                    rhs=xT_sb[:, 0, ts(0, 512)],
                    start=True,
                    stop=False,
                    tile_position=tp,
                    skip_group_check=True,
                )

            def emit_S(h, qi):
                tiles = []
                for kk in range(2):
                    s_ps = sp.tile([P, 1024], F32, tag="s", name="s_ps")
                    for kc in range(2):
                        nc.tensor.matmul(
                            s_ps[:, ts(kc, 512)],
                            lhsT=QT_sb[:, h, ts(qi, P)],
                            rhs=KT_sb[:, h, ds(kk * 1024 + kc * 512, 512)],
                            start=True,
                            stop=True,
                        )
                    tiles.append(s_ps)
                return tiles

            def emit_w(e_t, rb4_t, last):
                for j in range(4):
                    out, tp = w_region(j)
                    nc.tensor.matmul(
                        out,
                        lhsT=rb4_t,
                        rhs=e_t[:, ts(j, 512)],
                        start=False,
                        stop=last,
                        tile_position=tp,
                        skip_group_check=True,
                    )

            NSTRIPES = NH * QT_TILES
            pend_s = emit_S(0, 0)
            for i in range(NSTRIPES):
                h, qi = i // QT_TILES, i % QT_TILES
                e_t = ep.tile([P, N], BF, tag="e", name="e_t")
                zs = []
                act = None
                for kk, s_ps in enumerate(pend_s):
                    z_t = zp.tile([P, 1], F32, tag=f"z{kk}", name="z_t")
                    act = nc.scalar.activation(
                        out=e_t[:, ts(kk, 1024)],
                        in_=s_ps,
                        func=AF.Exp,
                        scale=inv_sqrt_d,
                        accum_out=z_t,
                    )
                    zs.append(z_t)
                if i == 0:
                    # non-critical DMAs, gated so they don't steal prologue
                    # HBM bandwidth from x^T / head-0 weights
                    gated = []
                    for h2 in range(1, NH):
                        gated.append(
                            nc.sync.dma_start(out=wqk_sb[:, h2], in_=wqkT_d[h2])
                        )
                    gated.append(
                        nc.sync.dma_start(
                            out=xtok_sb,
                            in_=xtok_d.rearrange("(t p) d -> p t d", p=P),
                        )
                    )
                    gated.append(
                        nc.sync.dma_start(
                            out=wv_sb, in_=wvT_d.rearrange("(t p) d -> p t d", p=P)
                        )
                    )
                    gated.append(
                        nc.sync.dma_start(
                            out=wo_sb, in_=woT_d.rearrange("(t p) o -> p t o", p=P)
                        )
                    )
                    for g in gated:
                        add_dep_helper(g.ins, act.ins, sync=True, reason="defer-dma")
                if i + 1 < NSTRIPES:
                    ni = i + 1
                    pend_s = emit_S(ni // QT_TILES, ni % QT_TILES)
                r_t = zp.tile([P, 1], F32, tag="r", name="r_t")
                nc.vector.tensor_add(r_t, zs[0], zs[1])
                nc.vector.reciprocal(r_t, r_t)
                # rb4 column h = r (bf16), other columns zero
                rb4_t = zp.tile([P, NH], BF, tag="rb", name="rb4_t")
                nc.vector.tensor_tensor(
                    rb4_t,
                    oneh_sb[:, h, :],
                    r_t.to_broadcast((P, NH)),
                    mybir.AluOpType.mult,
                )
                # the w accumulator bank is disjoint from the S ring, so
                # these never gate the exp pipeline
                emit_w(e_t, rb4_t, i == NSTRIPES - 1)
                # interleaved background projection work
                bg_advance(i)

            # ---------------- tail ----------------
            # w4_ps regions -> w4_sb [4, 2048] bf16
            for j in range(4):
                out, _ = w_region(j)
                nc.vector.tensor_copy(w4_sb[:, ts(j, 512)], out)
            # pipelined: transpose w4 chunk t -> wT4 (single-pass bf16), then
            # its two u matmuls (u = w @ x accumulated in two half-slots)
            u_ps = [
                sp.tile([P, 512], F32, tag="s", name="u_ps0"),
                sp.tile([P, 512], F32, tag="s", name="u_ps1"),
            ]
            for t in range(TOK_TILES):
                tpps = sp.tile([P, NH], BF, tag="s", name="tp_ps")
                nc.tensor.transpose(tpps, w4_sb[:, ts(t, P)], identB)
                nc.vector.tensor_copy(wT4_sb[:, t, :], tpps)
                for dc in range(2):
                    nc.tensor.matmul(
                        u_ps[dc][:NH, :],
                        lhsT=wT4_sb[:, t, :],
                        rhs=xtok_sb[:, t, ts(dc, 512)],
                        start=(t == 0),
                        stop=(t == TOK_TILES - 1),
                    )
            for dc in range(2):
                nc.vector.tensor_copy(u4_sb[:, ts(dc, 512)], u_ps[dc][:NH, :])
            # pipelined: transpose u chunk i -> uTz (block-diag scatter),
            # then its 4 att matmuls (att4 = u @ Wv^T)
            att4_ps = wp.tile([NH, P], F32, tag="w", name="att4_ps")
            for i in range(IT):
                tpps = pp.tile([P, NH], BF, tag="proj", name="tpu_ps")
                nc.tensor.transpose(tpps, u4_sb[:, ts(i, P)], identB)
                nc.vector.tensor_copy(
                    uTz_sb[:, i].rearrange("p a b -> p (a b)")[:, :: NH + 1],
                    tpps,
                )
                for hh in range(NH):
                    nc.tensor.matmul(
                        att4_ps,
                        lhsT=uTz_sb[:, i, hh, :],
                        rhs=wv_sb[:, i, ts(hh, HD)],
                        start=(i == 0 and hh == 0),
                        stop=(i == IT - 1 and hh == NH - 1),
                    )
            nc.vector.tensor_copy(att4_sb, att4_ps)
            attT_ps = sp.tile([P, NH], BF, tag="s", name="attT_ps")
            nc.tensor.transpose(attT_ps, att4_sb, identB)
            nc.vector.tensor_copy(attT_sb, attT_ps)
            p_ps = sp.tile([1, HID], F32, tag="s", name="p_ps")
            for oc in range(2):
                for hh in range(NH):
                    nc.tensor.matmul(
                        p_ps[:, ts(oc, 512)],
                        lhsT=attT_sb[:, hh : hh + 1],
                        rhs=wo_sb[:, hh, ts(oc, 512)],
                        start=(hh == 0),
                        stop=(hh == NH - 1),
                    )
            nc.vector.tensor_copy(pooled_sb, p_ps)
            nc.sync.dma_start(out=out_d, in_=pooled_sb)

    nc.finalize()  # Bacc: event-sem pass packs multi-waits into legal encodings
    return nc


def _get_nc():
    if "nc" not in _cache:
        _cache["nc"] = _build_nc()
    return _cache["nc"]


def _host_prep(inputs):
    """Build the 8 per-core input maps (host-side shard + transpose + cast)."""
    x = np.asarray(inputs["chunk_embeddings"], np.float32)
    in_maps = []
    for c in range(NCORES):
        b, hg = c // 2, c % 2
        sl = slice(hg * HGW, (hg + 1) * HGW)
        in_maps.append(
            {
                "xT": np.ascontiguousarray(x[b].T).astype(BF16),
                "xtok": np.ascontiguousarray(x[b]).astype(BF16),
                "wqkT": np.ascontiguousarray(
                    np.stack(
                        [
                            np.asarray(inputs[k], np.float32)[sl, :]
                            .T.reshape(HID, NH, HD)
                            .transpose(1, 0, 2)
                            for k in ("Wq", "Wk")
                        ]
                    )  # [2, NH, HID, HD]
                    .reshape(2, NH, IT, P, HD)
                    .transpose(1, 3, 0, 2, 4)  # [NH, P, 2, IT, HD]
                ).astype(BF16),
                "wvT": np.ascontiguousarray(
                    np.asarray(inputs["Wv"], np.float32)[sl, :].T
                ).astype(BF16),
                "woT": np.ascontiguousarray(
                    np.asarray(inputs["Wo"], np.float32)[:, sl].T / np.float32(N)
                ).astype(BF16),
                "bqk_col": np.ascontiguousarray(
                    np.stack(
                        [
                            np.asarray(inputs[k], np.float32)[sl].reshape(NH, P).T
                            for k in ("bq", "bk")
                        ],
                        axis=1,
                    )
                ),
            }
        )
    return in_maps


def _unshard(results, inputs):
    bo = np.asarray(inputs["bo"], np.float32)
    bv = np.asarray(inputs["bv"], np.float32)
    Wo = np.asarray(inputs["Wo"], np.float32)
    bv_wo = Wo @ bv  # exact fold of the V bias through the output projection
    out = np.zeros((B, HID), np.float32)
    for b in range(B):
        out[b] = (
            results[2 * b]["out_pooled"][0]
            + results[2 * b + 1]["out_pooled"][0]
            + bv_wo
            + bo
        )
    return out


def _reference_numpy(inputs):
    """Fallback for non-trivial attention masks (never hit for the spec'd
    all-ones mask): straight numpy port of the reference."""
    x = np.asarray(inputs["chunk_embeddings"], np.float32)
    mask = np.asarray(inputs["attention_mask"])
    b, n, hid = x.shape

    def proj(W, bias):
        y = x @ np.asarray(W, np.float32).T + np.asarray(bias, np.float32)
        return y.reshape(b, n, HEADS, HD).transpose(0, 2, 1, 3)

    Q = proj(inputs["Wq"], inputs["bq"])
    K = proj(inputs["Wk"], inputs["bk"])
    V = proj(inputs["Wv"], inputs["bv"])
    s = np.einsum("bhqd,bhkd->bhqk", Q, K) / np.float32(np.sqrt(HD))
    s = np.where(mask[:, None, None, :] == 0, np.float32(-1e9), s)
    s = s - s.max(axis=-1, keepdims=True)
    e = np.exp(s)
    a = e / e.sum(axis=-1, keepdims=True)
    att = np.einsum("bhqk,bhkd->bhqd", a, V)
    att = att.transpose(0, 2, 1, 3).reshape(b, n, hid)
    out = att @ np.asarray(inputs["Wo"], np.float32).T + np.asarray(
        inputs["bo"], np.float32
    )
    m = mask[:, :, None].astype(np.float32)
    return (out * m).sum(axis=1) / m.sum(axis=1)


def _run(inputs, trace=False):
    from concourse.bass_utils import run_bass_kernel_spmd

    nc = _get_nc()
    in_maps = _host_prep(inputs)
    res = run_bass_kernel_spmd(
        nc, in_maps, core_ids=list(range(NCORES)), trace=trace
    )
    _cache["last_result"] = res
    return _unshard(res.results, inputs)


def kernel(**inputs):
    mask = np.asarray(inputs["attention_mask"])
    if not np.all(mask == 1):
        return _reference_numpy(inputs)
    return _run(inputs, trace=False)


def kernel_traced(**inputs):
    """Like kernel() but with NTFF profiling; returns (out, exec_time_ns)."""
    out = _run(inputs, trace=True)
    return out, _cache["last_result"].exec_time_ns


# revision 18
# speedup vs baseline: 1.1056x; 1.0777x over previous
"""AttentionPooling Trainium2 kernel (8 NeuronCores, Bass/Tile).

Sharding: (batch, head-group) — core c handles batch b=c//2 and heads
4*(c%2)..4*(c%2)+3. Each core computes, for its 4 heads, Q^T/K^T (head-dim
major) projections, then a one-pass pooled attention over 64 query stripes
(4 heads x 16 stripes of 128 queries):

  S = Q_stripe K^T / sqrt(d)      (PE, bf16, 4 matmuls, two [128,1024] tiles)
  E = exp(S), Z0/Z1 = rowsums     (ScalarE: two 1024-wide ACTIVATEs + accum)
  r = 1/(Z0+Z1)                   (VectorE)
  w += r^T E                      (PE, 4 col-tiled matmuls, PSUM-accumulated)

PSUM budget (8 banks of 512 fp32): sp pool 3x[128,1024] f32 S half-tiles
(6 banks, 1.5-stripe-deep softmax pipeline), wp 1 bank (w accumulator,
k-chunk j at partition offset 32j via matmul column tile_position), pp 1
bank (projection chunks). The w/proj banks are disjoint from the S ring, so
the exp pipeline's only dependency is its own score matmuls.

The V projection is never materialized: attended_mean*N = (w @ x) @ Wv_h^T
(+ bv folded on the host), so the tail computes u = w @ x against
token-major x and u @ Wv^T, with single-pass bf16 transposes pipelined
against the u/att matmuls. The mean-pool is folded through the output
projection; V/output biases fold on the host:
  pooled = pooled_partial(core even) + pooled_partial(core odd) + Wo@bv + bo

Prologue: per-DMA issue is ~0.7us, so the critical set is 6 transfers
(combined head-0 Q+K weights, 4 x^T quarters, combined biases); head-0
projection matmuls are emitted grouped by contraction i-tile so they chase
the x^T quarters, and their evac+bias runs on the (otherwise idle) ScalarE.
All other DMAs are dependency-gated on the first ACTIVATE.
"""

import sys

import numpy as np

for _p in ("/opt/trn_rl_repo",):
    if _p not in sys.path:
        sys.path.append(_p)

import ml_dtypes

B, N, HID = 4, 2048, 1024
HEADS, HD = 8, 128
NH = 4          # heads per core
HGW = NH * HD   # head-group width (512)
NCORES = 8
P = 128
IT = HID // P   # 8 i-tiles
QT_TILES = N // P    # 16 query stripes
TOK_TILES = N // P   # 16 token tiles

BF16 = ml_dtypes.bfloat16

_cache = {}


def _build_nc():
    import concourse.bacc as bacc
    import concourse.tile as tile
    from concourse import mybir
    from concourse.bass import ds, ts
    from concourse.masks import make_identity
    from concourse.tile import add_dep_helper

    BF = mybir.dt.bfloat16
    F32 = mybir.dt.float32
    AF = mybir.ActivationFunctionType

    nc = bacc.Bacc(trn_type="TRN2")

    xT_d = nc.dram_tensor("xT", (HID, N), BF, kind="ExternalInput").ap()
    xtok_d = nc.dram_tensor("xtok", (N, HID), BF, kind="ExternalInput").ap()
    wqkT_d = nc.dram_tensor("wqkT", (NH, P, 2, IT, HD), BF, kind="ExternalInput").ap()
    wvT_d = nc.dram_tensor("wvT", (HID, HGW), BF, kind="ExternalInput").ap()
    woT_d = nc.dram_tensor("woT", (HGW, HID), BF, kind="ExternalInput").ap()
    bqk_d = nc.dram_tensor("bqk_col", (P, 2, NH), F32, kind="ExternalInput").ap()
    out_d = nc.dram_tensor("out_pooled", (1, HID), F32, kind="ExternalOutput").ap()

    inv_sqrt_d = float(1.0 / np.sqrt(HD))

    with tile.TileContext(nc) as tc:
        with (
            tc.tile_pool(name="persist", bufs=1) as persist,
            tc.tile_pool(name="sp", bufs=3, space="PSUM") as sp,
            tc.tile_pool(name="wp", bufs=1, space="PSUM") as wp,
            tc.tile_pool(name="pp", bufs=1, space="PSUM") as pp,
            tc.tile_pool(name="ep", bufs=3) as ep,
            tc.tile_pool(name="zp", bufs=4) as zp,
        ):
            # ---- critical-path DMAs (everything else is gated on act0) ----
            xT_sb = persist.tile([P, IT, N], BF)
            wqk_sb = persist.tile([P, NH, 2, IT, HD], BF)
            xT_r = xT_d.rearrange("(t p) n -> p t n", p=P)
            bqk_sb = persist.tile([P, 2, NH], F32)
            nc.sync.dma_start(out=wqk_sb[:, 0], in_=wqkT_d[0])
            for qq in range(4):
                nc.sync.dma_start(
                    out=xT_sb[:, 2 * qq : 2 * qq + 2, :],
                    in_=xT_r[:, 2 * qq : 2 * qq + 2, :],
                )
            nc.sync.dma_start(out=bqk_sb, in_=bqk_d)
            # tiles for the gated DMAs (emitted inside the stripe loop)
            xtok_sb = persist.tile([P, TOK_TILES, HID], BF)
            wv_sb = persist.tile([P, IT, HGW], BF)
            wo_sb = persist.tile([P, NH, HID], BF)

            identB = persist.tile([NH, NH], BF)
            make_identity(nc, identB)
            # one-hot columns: oneh_sb[p, h, h'] = 1.0 iff h == h'
            oneh_sb = persist.tile([P, NH, NH], BF)
            nc.vector.memset(oneh_sb, 0.0)
            for h in range(NH):
                nc.vector.memset(oneh_sb[:, h, h : h + 1], 1.0)
            zs4_sb = persist.tile([P, NH], BF)
            nc.vector.memset(zs4_sb, 0.0)

            QT_sb = persist.tile([P, NH, N], BF)
            KT_sb = persist.tile([P, NH, N], BF)
            w4_sb = persist.tile([NH, N], BF)
            wT4_sb = persist.tile([P, TOK_TILES, NH], BF)
            u4_sb = persist.tile([NH, HID], BF)
            # uTz[p, i, h, h'] = u_h[i*128+p] iff h' == h else 0 (block-diag
            # zero padding so per-head u@Wv^T matmuls share one accumulator)
            uTz_sb = persist.tile([P, IT, NH, NH], BF)
            nc.vector.memset(uTz_sb, 0.0)
            att4_sb = persist.tile([NH, P], BF)
            attT_sb = persist.tile([P, NH], BF)
            pooled_sb = persist.tile([1, HID], F32)

            # ---------------- prologue: head 0's K + first Q chunk --------
            # 5 parallel [128,512] chunk accumulators: 3 borrowed sp slots +
            # wp + pp. Matmuls grouped by i-tile chase the x^T quarters;
            # evac+bias on ScalarE (idle until the first exp).
            pro_ps = [
                sp.tile([P, 512], F32, tag="s", name="pro0"),
                sp.tile([P, 512], F32, tag="s", name="pro1"),
                sp.tile([P, 512], F32, tag="s", name="pro2"),
                wp.tile([P, 512], F32, tag="w", name="pro3"),
                pp.tile([P, 512], F32, tag="proj", name="pro4"),
            ]
            # (proj_i, chunk): K c0..c3 then Q c0
            PRO = ((1, 0), (1, 1), (1, 2), (1, 3), (0, 0))
            for i in range(IT):
                for (pi, c), ps in zip(PRO, pro_ps):
                    nc.tensor.matmul(
                        ps,
                        lhsT=wqk_sb[:, 0, pi, i, :],
                        rhs=xT_sb[:, i, ts(c, 512)],
                        start=(i == 0),
                        stop=(i == IT - 1),
                    )
            # evac order: Q first (stripe 0's lhsT), then K c0..c3
            for (pi, c), ps in [(PRO[4], pro_ps[4])] + list(zip(PRO[:4], pro_ps[:4])):
                dst = (QT_sb, KT_sb)[pi]
                nc.scalar.add(dst[:, 0, ts(c, 512)], ps, bqk_sb[:, pi, 0:1])

            # Background projection work: remaining heads' Q/K chunks in the
            # dedicated pp bank; generators yield mid-chunk for fine-grained
            # interleaving (safe: nothing else writes the pp bank, and the
            # pool serializes chunk n+1 behind chunk n's evacuation).
            def qk_chunk(proj_i, h, c):
                dst = (QT_sb, KT_sb)[proj_i]
                ps = pp.tile([P, 512], F32, tag="proj", name="ps_qk")
                for i in range(IT):
                    nc.tensor.matmul(
                        ps,
                        lhsT=wqk_sb[:, h, proj_i, i, :],
                        rhs=xT_sb[:, i, ts(c, 512)],
                        start=(i == 0),
                        stop=(i == IT - 1),
                    )
                    if i == 3:
                        yield
                nc.vector.tensor_scalar_add(
                    dst[:, h, ts(c, 512)], ps, bqk_sb[:, proj_i, h : h + 1]
                )
                yield

            bg_tasks = []
            for c in range(1, 4):
                bg_tasks.append(qk_chunk(0, 0, c))
            for h2 in range(1, NH):
                # K chunks first: head h2's stripes start at stripe 16*h2
                # and need ALL of K^T(h2) but only the first Q chunk.
                for c in range(4):
                    bg_tasks.append(qk_chunk(1, h2, c))
                for c in range(4):
                    bg_tasks.append(qk_chunk(0, h2, c))
            bg_tasks.reverse()
            BG_STEPS = 2 * len(bg_tasks)
            BG_SPREAD = 40  # finish all background work by stripe 40 of 64

            def bg_advance(si):
                lo = si * BG_STEPS // BG_SPREAD
                hi = min((si + 1) * BG_STEPS // BG_SPREAD, BG_STEPS)
                for _ in range(max(0, hi - lo)):
                    while bg_tasks:
                        try:
                            next(bg_tasks[-1])
                            break
                        except StopIteration:
                            bg_tasks.pop()

            # ---------------- pooled attention stripe loop ----------------
            # w accumulator: [128, 512] fp32 = 1 PSUM bank. k-chunk j lives
            # at partitions [32j, +4) (heads on rows +0..3) via matmul column
            # tile_position. Zero-matmuls open each sub-region's accumulation
            # group so later matmuls can all use start=False regardless of
            # has_written clear granularity.
            w4_ps = wp.tile([P, 512], F32, tag="w", name="w4_ps")

            def w_region(j):
                poff = 32 * j
                out = w4_ps[poff : poff + NH, :]
                tp = (0, poff) if poff else None
                return out, tp

            for j in range(4):
                out, tp = w_region(j)
                nc.tensor.matmul(
                    out,
                    lhsT=zs4_sb,
                    rhs=xT_sb[:, 0, ts(0, 512)],
                    start=True,
                    stop=False,
                    tile_position=tp,
                    skip_group_check=True,
                )

            def emit_S(h, qi):
                tiles = []
                for kk in range(2):
                    s_ps = sp.tile([P, 1024], F32, tag="s", name="s_ps")
                    for kc in range(2):
                        nc.tensor.matmul(
                            s_ps[:, ts(kc, 512)],
                            lhsT=QT_sb[:, h, ts(qi, P)],
                            rhs=KT_sb[:, h, ds(kk * 1024 + kc * 512, 512)],
                            start=True,
                            stop=True,
                        )
                    tiles.append(s_ps)
                return tiles

            def emit_w(e_t, rb4_t, last):
                for j in range(4):
                    out, tp = w_region(j)
                    nc.tensor.matmul(
                        out,
                        lhsT=rb4_t,
                        rhs=e_t[:, ts(j, 512)],
                        start=False,
                        stop=last,
                        tile_position=tp,
                        skip_group_check=True,
                    )

            NSTRIPES = NH * QT_TILES
            pend_s = emit_S(0, 0)
            for i in range(NSTRIPES):
                h, qi = i // QT_TILES, i % QT_TILES
                e_t = ep.tile([P, N], BF, tag="e", name="e_t")
                zs = []
                act = None
                for kk, s_ps in enumerate(pend_s):
                    z_t = zp.tile([P, 1], F32, tag=f"z{kk}", name="z_t")
                    act = nc.scalar.activation(
                        out=e_t[:, ts(kk, 1024)],
                        in_=s_ps,
                        func=AF.Exp,
                        scale=inv_sqrt_d,
                        accum_out=z_t,
                    )
                    zs.append(z_t)
                if i == 0:
                    # non-critical DMAs, gated so they don't steal prologue
                    # HBM bandwidth from x^T / head-0 weights
                    gated = []
                    for h2 in range(1, NH):
                        gated.append(
                            nc.sync.dma_start(out=wqk_sb[:, h2], in_=wqkT_d[h2])
                        )
                    gated.append(
                        nc.sync.dma_start(
                            out=xtok_sb,
                            in_=xtok_d.rearrange("(t p) d -> p t d", p=P),
                        )
                    )
                    gated.append(
                        nc.sync.dma_start(
                            out=wv_sb, in_=wvT_d.rearrange("(t p) d -> p t d", p=P)
                        )
                    )
                    gated.append(
                        nc.sync.dma_start(
                            out=wo_sb, in_=woT_d.rearrange("(t p) o -> p t o", p=P)
                        )
                    )
                    for g in gated:
                        add_dep_helper(g.ins, act.ins, sync=True, reason="defer-dma")
                if i + 1 < NSTRIPES:
                    ni = i + 1
                    pend_s = emit_S(ni // QT_TILES, ni % QT_TILES)
                r_t = zp.tile([P, 1], F32, tag="r", name="r_t")
                nc.vector.tensor_add(r_t, zs[0], zs[1])
                nc.vector.reciprocal(r_t, r_t)
                # rb4 column h = r (bf16), other columns zero
                rb4_t = zp.tile([P, NH], BF, tag="rb", name="rb4_t")
                nc.vector.tensor_tensor(
                    rb4_t,
                    oneh_sb[:, h, :],
                    r_t.to_broadcast((P, NH)),
                    mybir.AluOpType.mult,
                )
                # the w accumulator bank is disjoint from the S ring, so
                # these never gate the exp pipeline
                emit_w(e_t, rb4_t, i == NSTRIPES - 1)
                # interleaved background projection work
                bg_advance(i)

            # ---------------- tail ----------------
            # w4_ps regions -> w4_sb [4, 2048] bf16
            for j in range(4):
                out, _ = w_region(j)
                nc.vector.tensor_copy(w4_sb[:, ts(j, 512)], out)
            # pipelined: transpose w4 chunk t -> wT4 (single-pass bf16), then
            # its two u matmuls (u = w @ x accumulated in two half-slots)
            # u accumulators live in the w/proj banks (free after the
            # stripe loop) so all three sp slots stay available for the
            # transpose pipeline
            u_ps = [
                wp.tile([P, 512], F32, tag="w", name="u_ps0"),
                pp.tile([P, 512], F32, tag="proj", name="u_ps1"),
            ]
            for t in range(TOK_TILES):
                tpps = sp.tile([P, NH], BF, tag="s", name="tp_ps")
                nc.tensor.transpose(tpps, w4_sb[:, ts(t, P)], identB)
                nc.vector.tensor_copy(wT4_sb[:, t, :], tpps)
                for dc in range(2):
                    nc.tensor.matmul(
                        u_ps[dc][:NH, :],
                        lhsT=wT4_sb[:, t, :],
                        rhs=xtok_sb[:, t, ts(dc, 512)],
                        start=(t == 0),
                        stop=(t == TOK_TILES - 1),
                    )
            for dc in range(2):
                nc.vector.tensor_copy(u4_sb[:, ts(dc, 512)], u_ps[dc][:NH, :])
            # pipelined: transpose u chunk i -> uTz (block-diag scatter),
            # then its 4 att matmuls (att4 = u @ Wv^T)
            att4_ps = wp.tile([NH, P], F32, tag="w", name="att4_ps")
            for i in range(IT):
                tpps = sp.tile([P, NH], BF, tag="s", name="tpu_ps")
                nc.tensor.transpose(tpps, u4_sb[:, ts(i, P)], identB)
                nc.vector.tensor_copy(
                    uTz_sb[:, i].rearrange("p a b -> p (a b)")[:, :: NH + 1],
                    tpps,
                )
                for hh in range(NH):
                    nc.tensor.matmul(
                        att4_ps,
                        lhsT=uTz_sb[:, i, hh, :],
                        rhs=wv_sb[:, i, ts(hh, HD)],
                        start=(i == 0 and hh == 0),
                        stop=(i == IT - 1 and hh == NH - 1),
                    )
            nc.vector.tensor_copy(att4_sb, att4_ps)
            attT_ps = sp.tile([P, NH], BF, tag="s", name="attT_ps")
            nc.tensor.transpose(attT_ps, att4_sb, identB)
            nc.vector.tensor_copy(attT_sb, attT_ps)
            p_ps = sp.tile([1, HID], F32, tag="s", name="p_ps")
            for oc in range(2):
                for hh in range(NH):
                    nc.tensor.matmul(
                        p_ps[:, ts(oc, 512)],
                        lhsT=attT_sb[:, hh : hh + 1],
                        rhs=wo_sb[:, hh, ts(oc, 512)],
                        start=(hh == 0),
                        stop=(hh == NH - 1),
                    )
            nc.vector.tensor_copy(pooled_sb, p_ps)
            nc.sync.dma_start(out=out_d, in_=pooled_sb)

    nc.finalize()  # Bacc: event-sem pass packs multi-waits into legal encodings
    return nc


def _get_nc():
    if "nc" not in _cache:
        _cache["nc"] = _build_nc()
    return _cache["nc"]


def _host_prep(inputs):
    """Build the 8 per-core input maps (host-side shard + transpose + cast)."""
    x = np.asarray(inputs["chunk_embeddings"], np.float32)
    in_maps = []
    for c in range(NCORES):
        b, hg = c // 2, c % 2
        sl = slice(hg * HGW, (hg + 1) * HGW)
        in_maps.append(
            {
                "xT": np.ascontiguousarray(x[b].T).astype(BF16),
                "xtok": np.ascontiguousarray(x[b]).astype(BF16),
                "wqkT": np.ascontiguousarray(
                    np.stack(
                        [
                            np.asarray(inputs[k], np.float32)[sl, :]
                            .T.reshape(HID, NH, HD)
                            .transpose(1, 0, 2)
                            for k in ("Wq", "Wk")
                        ]
                    )  # [2, NH, HID, HD]
                    .reshape(2, NH, IT, P, HD)
                    .transpose(1, 3, 0, 2, 4)  # [NH, P, 2, IT, HD]
                ).astype(BF16),
                "wvT": np.ascontiguousarray(
                    np.asarray(inputs["Wv"], np.float32)[sl, :].T
                ).astype(BF16),
                "woT": np.ascontiguousarray(
                    np.asarray(inputs["Wo"], np.float32)[:, sl].T / np.float32(N)
                ).astype(BF16),
                "bqk_col": np.ascontiguousarray(
                    np.stack(
                        [
                            np.asarray(inputs[k], np.float32)[sl].reshape(NH, P).T
                            for k in ("bq", "bk")
                        ],
                        axis=1,
                    )
                ),
            }
        )
    return in_maps


def _unshard(results, inputs):
    bo = np.asarray(inputs["bo"], np.float32)
    bv = np.asarray(inputs["bv"], np.float32)
    Wo = np.asarray(inputs["Wo"], np.float32)
    bv_wo = Wo @ bv  # exact fold of the V bias through the output projection
    out = np.zeros((B, HID), np.float32)
    for b in range(B):
        out[b] = (
            results[2 * b]["out_pooled"][0]
            + results[2 * b + 1]["out_pooled"][0]
            + bv_wo
            + bo
        )
    return out


def _reference_numpy(inputs):
    """Fallback for non-trivial attention masks (never hit for the spec'd
    all-ones mask): straight numpy port of the reference."""
    x = np.asarray(inputs["chunk_embeddings"], np.float32)
    mask = np.asarray(inputs["attention_mask"])
    b, n, hid = x.shape

    def proj(W, bias):
        y = x @ np.asarray(W, np.float32).T + np.asarray(bias, np.float32)
        return y.reshape(b, n, HEADS, HD).transpose(0, 2, 1, 3)

    Q = proj(inputs["Wq"], inputs["bq"])
    K = proj(inputs["Wk"], inputs["bk"])
    V = proj(inputs["Wv"], inputs["bv"])
    s = np.einsum("bhqd,bhkd->bhqk", Q, K) / np.float32(np.sqrt(HD))
    s = np.where(mask[:, None, None, :] == 0, np.float32(-1e9), s)
    s = s - s.max(axis=-1, keepdims=True)
    e = np.exp(s)
    a = e / e.sum(axis=-1, keepdims=True)
    att = np.einsum("bhqk,bhkd->bhqd", a, V)
    att = att.transpose(0, 2, 1, 3).reshape(b, n, hid)
    out = att @ np.asarray(inputs["Wo"], np.float32).T + np.asarray(
        inputs["bo"], np.float32
    )
    m = mask[:, :, None].astype(np.float32)
    return (out * m).sum(axis=1) / m.sum(axis=1)


def _run(inputs, trace=False):
    from concourse.bass_utils import run_bass_kernel_spmd

    nc = _get_nc()
    in_maps = _host_prep(inputs)
    res = run_bass_kernel_spmd(
        nc, in_maps, core_ids=list(range(NCORES)), trace=trace
    )
    _cache["last_result"] = res
    return _unshard(res.results, inputs)


def kernel(**inputs):
    mask = np.asarray(inputs["attention_mask"])
    if not np.all(mask == 1):
        return _reference_numpy(inputs)
    return _run(inputs, trace=False)


def kernel_traced(**inputs):
    """Like kernel() but with NTFF profiling; returns (out, exec_time_ns)."""
    out = _run(inputs, trace=True)
    return out, _cache["last_result"].exec_time_ns
